# revision 1
# baseline (speedup 1.0000x reference)
"""Trainium2 Bass kernel for nn_DisplacementField (tri-plane nearest-neighbor
embedding lookup).

Reference semantics: for each of N=1M points with coords (x,y,z) and time
t01 in [0,1):
    t  = 2*t01 - 1;  p = -pts / 1.6
    ix   = round(((t   + 1) * 0.5) * 127)            in [0,127]
    iy_a = clip(round(((p_a + 1) * 0.5) * 511), 0, 511)
    feat = prod_a plane_a[:, iy_a, ix]               -> [N, 32]
feature_A/feature_B both == feat except (possibly) the last row (the
reference shifts only data[-1]); that row and the scalar cond select are
fixed on the host in exact f32 numpy.

Device strategy (8 cores, data-parallel over N):
  - planes repacked host-side to [H*W/2, 64] f32 "pair tables": row
    r = iy*64 + (ix>>1) holds the 128B vectors for ix even|odd. Row index
    fits int16 (<= 32767) as required by the SWDGE dma_gather ISA, whose
    elem_size must be a 256B multiple.
  - per chunk of 128*C points: DVE computes (bit-exact IEEE f32, round-half
    -even via the +2^23 magic trick) the pair-row index and the shared
    ix-parity bit; PE selection matmuls fold the [128,C] row indices into
    the gather ISA's wrapped+replicated [128, 8C] int16 layout (index i of
    the gather list lives at partition i%16 (all 8 groups), slot i//16);
    dma_gather fetches 256B/point/plane; DVE multiplies the three pair
    rows and selects the 128B half by parity; result stored as [*,32] f32.
  - out point order is partition-minor (point i -> partition i%128,
    slot i//128) as dictated by dma_gather; the host permutes shards
    to/from that order (part of sharding/unsharding).
All device arithmetic is bit-identical to the f32 reference chain.
"""

import numpy as np

N = 1_000_000
FEAT = 32
RES_H = 512
RES_W = 128
BOUNDS = 1.6
TIME_STEP = 1.0 / (2.0 * RES_W)
NCORES = 8

# per-core layout: 128 partitions x J points, processed in NCHUNK chunks of
# C slots; 8C f32 = one PSUM bank (C <= 64)
J = 992
C = 62
NCHUNK = J // C            # 16
NPC = 128 * J              # 126,976 points per core
NPAD = NPC * NCORES        # 1,015,808

MAGIC = 8388608.0          # 2^23: x+MAGIC-MAGIC == round-half-even(x), 0<=x<2^22

_CACHE = {}


def _build_nc():
    from concourse import bass, bacc, mybir
    import concourse.tile as tile

    f32 = mybir.dt.float32
    i16 = mybir.dt.int16
    i32 = mybir.dt.int32
    Alu = mybir.AluOpType

    # 4 SWDGE queues: descriptor generation for concurrent dma_gathers runs
    # in parallel gen contexts (measured ~2.9ns/idx vs 7.8ns single-queue)
    nc = bacc.Bacc("TRN2", target_bir_lowering=False, num_swdge_queues=4)
    pn = nc.dram_tensor("pnorm", [128, J, 3], f32, kind="ExternalInput")
    tm = nc.dram_tensor("time_in", [128, J], f32, kind="ExternalInput")
    sel_in = nc.dram_tensor("sel_in", [128, 8 * 128], f32, kind="ExternalInput")
    tabs = [
        nc.dram_tensor(f"tab{a}", [RES_H * RES_W // 2, 2 * FEAT], f32,
                       kind="ExternalInput")
        for a in range(3)
    ]
    feat = nc.dram_tensor("feat", [128, J, FEAT], f32, kind="ExternalOutput")

    NIDX = 128 * C

    with tile.TileContext(nc) as tc:
        with (
            tc.tile_pool(name="const", bufs=1) as cp,
            tc.tile_pool(name="io", bufs=4) as io,
            tc.tile_pool(name="g", bufs=2) as gp,
            tc.tile_pool(name="tmp", bufs=2) as tp,
            tc.tile_pool(name="ps", bufs=2, space="PSUM") as pp,
        ):
            sel = cp.tile([128, 8 * 128], f32)
            nc.sync.dma_start(out=sel[:], in_=sel_in[:])

            for k in range(NCHUNK):
                sl = slice(k * C, (k + 1) * C)
                p3 = io.tile([128, C, 3], f32)
                t0 = io.tile([128, C], f32)
                nc.sync.dma_start(out=p3[:], in_=pn[:, sl, :])
                nc.scalar.dma_start(out=t0[:], in_=tm[:, sl])

                # ---- time path: fx = rhe(((t+1)*0.5)*127), t = 2*t01-1.
                # Fused tensor_scalar op pairs are chosen so the result is
                # identical whether or not the intermediate rounds to f32
                # (2nd op is an exact pow2 scale / exact add / min/max /
                # Sterbenz subtract).
                t2 = tp.tile([128, C], f32)
                nc.vector.tensor_scalar(
                    out=t2[:], in0=t0[:], scalar1=2.0, scalar2=1.0,
                    op0=Alu.mult, op1=Alu.subtract)
                u1 = tp.tile([128, C], f32)
                nc.vector.tensor_scalar(
                    out=u1[:], in0=t2[:], scalar1=1.0, scalar2=0.5,
                    op0=Alu.add, op1=Alu.mult)
                u2 = tp.tile([128, C], f32)
                nc.vector.tensor_scalar_mul(u2[:], u1[:], 127.0)
                mx = tp.tile([128, C], f32)
                nc.vector.tensor_scalar_add(mx[:], u2[:], MAGIC)
                fx = tp.tile([128, C], f32)
                nc.vector.tensor_scalar_sub(fx[:], mx[:], MAGIC)
                # qr = ix>>1 = rhe(fx*0.5 - 0.25)  (both ops exact)
                q1 = tp.tile([128, C], f32)
                nc.vector.tensor_scalar(
                    out=q1[:], in0=fx[:], scalar1=0.5, scalar2=-0.25,
                    op0=Alu.mult, op1=Alu.add)
                mq = tp.tile([128, C], f32)
                nc.vector.tensor_scalar_add(mq[:], q1[:], MAGIC)
                qr = tp.tile([128, C], f32)
                nc.vector.tensor_scalar_sub(qr[:], mq[:], MAGIC)
                # parity bit = fx - 2*qr  (exact)
                tb = tp.tile([128, C], f32)
                nc.vector.tensor_scalar_mul(tb[:], qr[:], 2.0)
                bitf = tp.tile([128, C], f32)
                nc.vector.tensor_tensor(
                    out=bitf[:], in0=fx[:], in1=tb[:], op=Alu.subtract)
                bit = tp.tile([128, C], i32, tag="bit", bufs=8)
                nc.vector.tensor_copy(bit[:], bitf[:])

                gs = []
                for a in range(3):
                    # iy: v = ((p+1)*0.5)*511, clip, rhe; row = iy*64 + qr
                    v1 = tp.tile([128, C], f32, tag=f"v1_{a}")
                    nc.vector.tensor_scalar(
                        out=v1[:], in0=p3[:, :, a], scalar1=1.0, scalar2=0.5,
                        op0=Alu.add, op1=Alu.mult)
                    v2 = tp.tile([128, C], f32, tag=f"v2_{a}")
                    nc.vector.tensor_scalar(
                        out=v2[:], in0=v1[:], scalar1=511.0, scalar2=0.0,
                        op0=Alu.mult, op1=Alu.max)
                    m3 = tp.tile([128, C], f32, tag=f"m3_{a}")
                    nc.vector.tensor_scalar(
                        out=m3[:], in0=v2[:], scalar1=511.0, scalar2=MAGIC,
                        op0=Alu.min, op1=Alu.add)
                    f64 = tp.tile([128, C], f32, tag=f"f64_{a}")
                    nc.vector.tensor_scalar(
                        out=f64[:], in0=m3[:], scalar1=MAGIC, scalar2=64.0,
                        op0=Alu.subtract, op1=Alu.mult)
                    rowf = tp.tile([128, C], f32, tag=f"rowf_{a}")
                    nc.vector.tensor_tensor(
                        out=rowf[:], in0=f64[:], in1=qr[:], op=Alu.add)

                    # fold [128,C] row indices into wrapped [128, 8C] int16:
                    # 8 selection matmuls (psum[:, b, :] = rows b*16..b*16+15
                    # of rowf replicated to all 8 partition groups), then one
                    # strided copy interleaving (b, a) -> slot a*8+b.
                    ps = pp.tile([128, 8, C], f32, tag=f"ps_{a}")
                    for b in range(8):
                        nc.tensor.matmul(
                            out=ps[:, b, :],
                            lhsT=sel[:, b * 128:(b + 1) * 128],
                            rhs=rowf[:],
                            start=True, stop=True)
                    wrapped = tp.tile([128, 8 * C], i16, tag=f"w_{a}", bufs=8)
                    wr_view = bass.AP(
                        wrapped.tensor, wrapped[:].offset,
                        [wrapped[:].ap[0], (1, 8), (8, C)])
                    nc.vector.tensor_copy(wr_view, ps[:])

                    g = gp.tile([128, C, 2 * FEAT], f32, tag="g", bufs=5)
                    nc.gpsimd.dma_gather(
                        out_ap=g[:],
                        in_ap=tabs[a][:],
                        idxs_ap=wrapped[:],
                        num_idxs=NIDX,
                        num_idxs_reg=NIDX,
                        elem_size=2 * FEAT,
                        single_packet=False,
                        queue_num=(k * 3 + a) % 4,
                    )
                    gs.append(g)

                # product on 64-wide pairs, then select the 128B half by the
                # shared ix-parity bit (in place), compact, store
                nc.vector.tensor_tensor(
                    out=gs[0][:], in0=gs[0][:], in1=gs[1][:], op=Alu.mult)
                nc.vector.tensor_tensor(
                    out=gs[0][:], in0=gs[0][:], in1=gs[2][:], op=Alu.mult)
                pred = bit[:, :, None].to_broadcast([128, C, FEAT])
                nc.vector.copy_predicated(
                    out=gs[0][:, :, 0:FEAT], mask=pred,
                    data=gs[0][:, :, FEAT:2 * FEAT])
                fc = tp.tile([128, C, FEAT], f32, tag="fc")
                nc.vector.tensor_copy(fc[:], gs[0][:, :, 0:FEAT])
                nc.sync.dma_start(out=feat[:, sl, :], in_=fc[:])

    # Tile assigns DMASW completion sems round-robin in *scheduled* order,
    # and the SWDGE ucode requires each DMASW sem to be driven by a single
    # queue. Re-derive queue_num from the assigned sem so sem i belongs to
    # queue i%4 always.
    import re
    for blk in nc.main_func.blocks:
        for ins in blk.instructions:
            if isinstance(ins, mybir.InstDMAGatherAnt) and ins.sync_info:
                for u in ins.sync_info.on_update:
                    m = re.match(r"DMASW(\d+)_", getattr(u, "ant_name", "") or "")
                    if m:
                        ins.queue_num = int(m.group(1)) % 4
    nc.finalize()
    return nc


def _get_nc():
    if "nc" not in _CACHE:
        _CACHE["nc"] = _build_nc()
    return _CACHE["nc"]


def _make_sel():
    # sel_in[p, b*128 + p'] = 1 iff p == b*16 + (p' % 16)
    sel = np.zeros((128, 8, 128), dtype=np.float32)
    p = np.arange(128)
    for b in range(8):
        for pp_ in range(128):
            sel[b * 16 + (pp_ % 16), b, pp_] = 1.0
    return sel.reshape(128, 8 * 128)


def _pack_tables(planes):
    # [F,H,W] -> [H*W, F] -> pair view [H*W/2, 2F]; row iy*64+(ix>>1)
    return [
        np.ascontiguousarray(
            np.asarray(p, dtype=np.float32).transpose(1, 2, 0)
        ).reshape(RES_H * RES_W // 2, 2 * FEAT)
        for p in planes
    ]


def _host_feat_row(prow, trow, planes):
    """Exact f32 replication of the reference gather/product for one point."""
    one = np.float32(1.0)
    half = np.float32(0.5)
    acc = np.float32(1.0)
    for a, plane in enumerate(planes):
        u = ((trow + one) * half) * np.float32(RES_W - 1)
        ix = int(np.clip(np.round(u).astype(np.int32), 0, RES_W - 1))
        v = ((prow[a] + one) * half) * np.float32(RES_H - 1)
        iy = int(np.clip(np.round(v).astype(np.int32), 0, RES_H - 1))
        acc = (acc * plane[:, iy, ix].astype(np.float32)).astype(np.float32)
    return acc


def _make_in_maps(pnorm, t01, planes):
    pn_pad = np.zeros((NPAD, 3), dtype=np.float32)
    pn_pad[:N] = pnorm
    t_pad = np.zeros(NPAD, dtype=np.float32)
    t_pad[:N] = t01

    tabs = _pack_tables(planes)
    sel = _make_sel()

    in_maps = []
    for c in range(NCORES):
        s = slice(c * NPC, (c + 1) * NPC)
        # device point order is partition-minor: point i -> (i%128, i//128)
        pn_dev = np.ascontiguousarray(
            pn_pad[s].reshape(J, 128, 3).transpose(1, 0, 2))
        t_dev = np.ascontiguousarray(t_pad[s].reshape(J, 128).T)
        in_maps.append({
            "pnorm": pn_dev,
            "time_in": t_dev,
            "sel_in": sel,
            "tab0": tabs[0],
            "tab1": tabs[1],
            "tab2": tabs[2],
        })
    return in_maps


def _device_feat(pnorm, t01, planes, trace=False, **kw):
    """Run the 8-core device kernel; returns (feat[:N], BassKernelResults)."""
    from concourse.bass_utils import run_bass_kernel_spmd

    in_maps = _make_in_maps(pnorm, t01, planes)
    nc = _get_nc()
    res = run_bass_kernel_spmd(nc, in_maps, list(range(NCORES)), trace=trace, **kw)
    feat = np.empty((NPAD, FEAT), dtype=np.float32)
    for c in range(NCORES):
        # undo partition-minor order
        feat[c * NPC:(c + 1) * NPC] = (
            res.results[c]["feat"].transpose(1, 0, 2).reshape(NPC, FEAT))
    return feat[:N], res


def kernel(pts, time, plane0, plane1, plane2):
    pts = np.asarray(pts, dtype=np.float32)
    time = np.asarray(time, dtype=np.float32)
    planes = tuple(np.asarray(p, dtype=np.float32) for p in (plane0, plane1, plane2))

    # host: exact f32 normalization (single IEEE divide, matches XLA bitwise)
    pnorm = np.divide(np.negative(pts), np.float32(BOUNDS), dtype=np.float32)
    t01 = time[:, 0]

    feat_orig, _ = _device_feat(pnorm, t01, planes)

    # host fix-up for the reference's last-row shift quirk (exact f32)
    ts32 = np.float32(TIME_STEP)
    p_last = pnorm[-1].copy()
    t_last = np.float32(time[-1, 0] * np.float32(2.0) - np.float32(1.0))
    p_shift = (p_last - ts32).astype(np.float32)
    t_shift = np.float32(t_last - ts32)
    shift_row = _host_feat_row(p_shift, t_shift, planes)

    cond = bool(p_last[0] + ts32 > np.float32(1.0))

    feature_A = feat_orig
    feature_B = feat_orig.copy()
    if cond:
        feature_A = feature_A.copy()
        feature_A[-1] = shift_row
    else:
        feature_B[-1] = shift_row
    return feature_A, feature_B

